# revision 12
# baseline (speedup 1.0000x reference)
"""Trainium2 Bass kernel for DigitConvolutionalModel (conv3x3 -> 3-layer MLP).

Strategy:
  - Pure data parallel over 8 NeuronCores: batch 65536 -> 8192 per core.
  - Host folds the 3x3 valid conv (28x28 -> 26x26) into W1:
        h1 = relu(conv(x) @ W1 + b1) = relu(x @ (C @ W1) + b1)
    where C (784, 676) is the sparse conv unfold matrix. W1f = C @ W1 is
    computed on host in float64 and cast down.
  - Host pre-transposes each x shard to (784, 8192) so the contraction dim
    lies on SBUF partitions; all three layers run in transposed layout
    (h^T = W^T @ x^T), so activations stay [feat_part, batch_free] and no
    on-chip transposes are needed. The (10, 8192) output is transposed back
    on host.
  - All matmul operands are bf16 (same 1 PE cycle/row as float32r but half
    the DMA bytes and half the LDWEIGHTS time); PSUM accumulation is fp32
    and bias+ReLU are fused into ScalarEngine activation reading PSUM.
    Measured end-to-end rel err ~3e-3 vs the 2e-2 gate.
  - Layers are software-pipelined across 512-column batch chunks
    (L1(c) | L2(c-1) | L3(c-2)); L1 runs k-outer (each x k-tile feeds 4
    matmuls the moment it lands, which also relaxes the deadline for the
    late k-tiles of each DMA group by most of a chunk).
  - x is fetched per-k ([128, cols] tiles: good DRAM locality, ~0.7us
    descriptor-gen each): chunks 0-1 as singles for startup latency, then
    4-chunk groups (4KB per-partition runs, ~400 GB/s).
  - W3 is padded on host to [128, 2, 128]: matmuls with a 10-wide output
    tile measured ~310ns vs 216ns for 128-wide (PE column-tiling mode);
    padding M3 to 128 keeps the PE in full-array mode.
  - A few dummy warmup matmuls (into the otherwise-idle ps2 banks, gated
    only on a gpsimd memset issued before the weight loads) ramp the PE
    p-state during the DMA lead-in.
"""

import os
import sys

sys.path.insert(0, "/opt/trn_rl_repo")

import numpy as np
import ml_dtypes

import concourse.bass as bass
import concourse.tile as tile
from concourse import mybir
import bass_rust
from concourse.bass_utils import run_bass_kernel_spmd

NCORES = 8
B = 65536
BC = B // NCORES          # 8192 rows per core
CHUNK = 512               # moving-dim tile (one PSUM bank of fp32)
NCHUNK = BC // CHUNK      # 16
NSINGLE = 4               # leading chunks loaded per-k individually
GROUP = 4                 # trailing chunks loaded GROUP at a time (per-k)

K1, NK1, K1T = 784, 7, 128     # L1 contraction tiling (K padded 784 -> 896)
K1P = NK1 * K1T                # 896
M1, NM1, M1T = 500, 4, 125     # L1 output-feature tiling
K2, NK2, K2T = 500, 4, 125
M2, NM2, M2T = 200, 2, 100
K3, NK3, K3T = 200, 2, 128     # L3 contraction padded 100 -> 128/tile
M3, M3P = 10, 128              # L3 output padded 10 -> 128

F32 = mybir.dt.float32
MM_DT = mybir.dt.bfloat16
NP_DT = ml_dtypes.bfloat16
N_WARMUP = int(os.environ.get("KERNEL_WARMUP", "8"))


def _split_excess_waits(nc, max_waits=1):
    """This walrus build caps sync-wait commands per instruction (Drain at 1).
    Hoist extra waits onto wait-only nops inserted just before, same engine."""
    ctr = 0
    for f in nc.m.functions:
        for bb in f.blocks:
            insts = bb.instructions
            i = 0
            while i < len(insts):
                inst = insts[i]
                si = inst.sync_info
                waits = list(si.on_wait) if (si and si.on_wait) else []
                if len(waits) > max_waits:
                    keep = waits[-max_waits:]
                    extra = waits[:-max_waits]
                    inst.sync_info = bass_rust.SyncInfo(
                        on_wait=keep, on_update=list(si.on_update or []))
                    nops = []
                    for j in range(0, len(extra), max_waits):
                        nop = mybir.InstNoOp(
                            name=f"WSPLIT-{ctr}", ins=[], outs=[])
                        ctr += 1
                        nop.engine = inst.engine
                        nop.sync_info = bass_rust.SyncInfo(
                            on_wait=extra[j:j + max_waits], on_update=[])
                        nops.append(nop)
                    insts[i:i] = nops
                    i += len(nops)
                i += 1
    return ctr


def build_bass():
    nc = bass.Bass(target_bir_lowering=False)
    Relu = mybir.ActivationFunctionType.Relu
    Ident = mybir.ActivationFunctionType.Identity

    xh = nc.declare_dram_parameter("xh", [NK1, K1T, BC], MM_DT, isOutput=False)
    w1 = nc.declare_dram_parameter("w1", [K1P, M1], MM_DT, isOutput=False)
    b1 = nc.declare_dram_parameter("b1", [M1], F32, isOutput=False)
    w2 = nc.declare_dram_parameter("w2", [K2, M2], MM_DT, isOutput=False)
    b2 = nc.declare_dram_parameter("b2", [M2], F32, isOutput=False)
    # w3 host-padded: [K3T, NK3, M3P]; rows 100.. and cols 10.. are zero
    w3 = nc.declare_dram_parameter("w3", [K3T, NK3, M3P], MM_DT, isOutput=False)
    b3 = nc.declare_dram_parameter("b3", [M3], F32, isOutput=False)
    out = nc.declare_dram_parameter("out", [M3, BC], F32, isOutput=True)

    with tile.TileContext(nc) as tc:
        with (
            tc.tile_pool(name="singles", bufs=1) as singles,
            tc.tile_pool(name="xp", bufs=2) as xp,
            tc.tile_pool(name="h1p", bufs=2) as h1p,
            tc.tile_pool(name="h2p", bufs=2) as h2p,
            tc.tile_pool(name="op", bufs=2) as op,
            tc.tile_pool(name="ps1p", bufs=4, space="PSUM") as ps1p,
            tc.tile_pool(name="ps2p", bufs=2, space="PSUM") as ps2p,
            tc.tile_pool(name="ps3p", bufs=2, space="PSUM") as ps3p,
        ):
            # warmup-source memset FIRST on gpsimd (so the warmup matmuls
            # aren't gated behind weight-load descriptor generation)
            warm = singles.tile([K1T, CHUNK], MM_DT)
            if N_WARMUP:
                nc.gpsimd.memset(warm, 0.0)

            # weights + biases via SWDGE (gpsimd) so they don't serialize
            # behind / ahead of the x prefetch stream on Sync; w1 per-k so
            # the first k-tile lands in ~1us of descriptor-gen
            w1_k = w1.rearrange("(k p) m -> p k m", k=NK1)
            w1_s = singles.tile([K1T, NK1, M1], MM_DT)
            nc.gpsimd.dma_start(out=w1_s[:, 0:1, :], in_=w1_k[:, 0:1, :])
            nc.gpsimd.dma_start(out=w1_s[:, 1:4, :], in_=w1_k[:, 1:4, :])
            nc.gpsimd.dma_start(out=w1_s[:, 4:7, :], in_=w1_k[:, 4:7, :])
            w1_ks = [w1_s[:, k, :] for k in range(NK1)]
            b1_s = singles.tile([M1T, NM1], F32)
            nc.gpsimd.dma_start(out=b1_s, in_=b1.rearrange("(m p) -> p m", m=NM1))
            w2_s = singles.tile([K2T, NK2, M2], MM_DT)
            nc.gpsimd.dma_start(out=w2_s, in_=w2.rearrange("(k p) m -> p k m", k=NK2))
            w3_s = singles.tile([K3T, NK3, M3P], MM_DT)
            nc.gpsimd.dma_start(out=w3_s, in_=w3[:, :, :])
            b2_s = singles.tile([M2T, NM2], F32)
            nc.gpsimd.dma_start(out=b2_s, in_=b2.rearrange("(m p) -> p m", m=NM2))
            b3_s = singles.tile([M3, 1], F32)
            nc.gpsimd.dma_start(out=b3_s, in_=b3.rearrange("(m p) -> p m", m=1))

            # Dummy activations on the warm tile: the compiler inserts the
            # ACT_TABLE_LOAD right before the first activation instruction
            # in queue order, where it would otherwise sit behind the first
            # real RELU's dependency waits (~20us) and add 1.3us to the
            # critical path. These run at ~8us instead.
            warm_act = singles.tile([K1T, 8], F32)
            nc.scalar.activation(out=warm_act, in_=warm[:, :8],
                                 func=Relu, bias=0.0, scale=1.0)
            nc.scalar.activation(out=warm_act, in_=warm[:, :8],
                                 func=Ident, bias=0.0, scale=1.0)

            # PE p-state warmup into the (idle at startup) ps2 banks
            for i in range(N_WARMUP):
                ps_w = ps2p.tile([K1T, CHUNK], F32, name="ps_w", tag="ps2")
                nc.tensor.matmul(ps_w, lhsT=warm[:, :K1T], rhs=warm,
                                 start=True, stop=True)

            # x prefetch: per-k [128, G*512] HWDGE loads, split across both
            # HWDGE sequencers (descriptor-gen is ~0.7us each, serialized
            # per sequencer)
            groups = [(c, 1) for c in range(NSINGLE)] + \
                [(c, min(GROUP, NCHUNK - c))
                 for c in range(NSINGLE, NCHUNK, GROUP)]
            chunk_src = {}  # chunk -> (ktile list, column offset)
            for g0, glen in groups:
                xks = []
                for k in range(NK1):
                    t = xp.tile([K1T, glen * CHUNK], MM_DT,
                                name=f"x{g0}_{k}",
                                tag=(f"xs_{k}" if glen == 1 else f"xg_{k}"),
                                bufs=(NSINGLE if glen == 1 else 2))
                    eng = nc.sync if glen == 1 else nc.gpsimd
                    eng.dma_start(
                        out=t, in_=xh[k, :, g0 * CHUNK:(g0 + glen) * CHUNK])
                    xks.append(t)
                for cc in range(g0, g0 + glen):
                    chunk_src[cc] = (xks, (cc - g0) * CHUNK)

            h1_tiles = [None] * NCHUNK
            h2_tiles = [None] * NCHUNK
            for c in range(NCHUNK + 2):
                # stage 1: L1 matmuls (k-outer) + relu for chunk c
                if c < NCHUNK:
                    xks, xoff = chunk_src[c]
                    ps1s = [ps1p.tile([M1T, CHUNK], F32, name=f"ps1_{m}",
                                      tag="ps1") for m in range(NM1)]
                    for k in range(NK1):
                        rhs = xks[k][:, xoff:xoff + CHUNK]
                        for m in range(NM1):
                            nc.tensor.matmul(
                                ps1s[m],
                                lhsT=w1_ks[k][:, m * M1T:(m + 1) * M1T],
                                rhs=rhs,
                                start=(k == 0), stop=(k == NK1 - 1))
                    h1s = []
                    for m in range(NM1):
                        h1 = h1p.tile([M1T, CHUNK], MM_DT, tag=f"h1_{m}")
                        nc.scalar.activation(
                            out=h1, in_=ps1s[m], func=Relu,
                            bias=b1_s[:, m:m + 1], scale=1.0)
                        h1s.append(h1)
                    h1_tiles[c] = h1s
                # stage 2: L2 for chunk c-1
                if 1 <= c <= NCHUNK:
                    h1s = h1_tiles[c - 1]
                    h2s = []
                    for m in range(NM2):
                        ps2 = ps2p.tile([M2T, CHUNK], F32, name="ps2",
                                        tag="ps2")
                        for k in range(NK2):
                            nc.tensor.matmul(
                                ps2,
                                lhsT=w2_s[:, k, m * M2T:(m + 1) * M2T],
                                rhs=h1s[k],
                                start=(k == 0), stop=(k == NK2 - 1))
                        # h2 padded to 128 partitions; zeroed once per
                        # physical buffer (c-1 < 2), RELU rewrites 0..99
                        h2 = h2p.tile([K3T, CHUNK], MM_DT, tag=f"h2_{m}")
                        if c - 1 < 2:
                            nc.vector.memset(h2, 0.0)
                        nc.scalar.activation(
                            out=h2[:M2T, :], in_=ps2, func=Relu,
                            bias=b2_s[:, m:m + 1], scale=1.0)
                        h2s.append(h2)
                    h2_tiles[c - 1] = h2s
                # stage 3: L3 for chunk c-2 + store
                if c >= 2:
                    cc = c - 2
                    h2s = h2_tiles[cc]
                    ps3 = ps3p.tile([M3P, CHUNK], F32)
                    for k in range(NK3):
                        nc.tensor.matmul(
                            ps3, lhsT=w3_s[:, k, :], rhs=h2s[k],
                            start=(k == 0), stop=(k == NK3 - 1))
                    o_t = op.tile([M3, CHUNK], F32)
                    nc.scalar.activation(
                        out=o_t, in_=ps3[:M3, :], func=Ident,
                        bias=b3_s[:, 0:1], scale=1.0)
                    nc.sync.dma_start(
                        out=out[:, cc * CHUNK:(cc + 1) * CHUNK], in_=o_t)

    _split_excess_waits(nc)
    return nc


_NC_CACHE = None


def _get_nc():
    global _NC_CACHE
    if _NC_CACHE is None:
        _NC_CACHE = build_bass()
    return _NC_CACHE


def _conv_unfold(conv_w):
    """C (784, 676): x_flat @ C == flatten(valid 3x3 xcorr of x as 28x28)."""
    C = np.zeros((784, 676), dtype=np.float64)
    w = np.asarray(conv_w, dtype=np.float64)
    for i in range(26):
        for j in range(26):
            q = 26 * i + j
            for di in range(3):
                for dj in range(3):
                    C[28 * (i + di) + (j + dj), q] += w[di, dj]
    return C


def kernel(x, conv_w, W1, b1, W2, b2, W3, b3, _trace=False, _tmpdir=None):
    x = np.asarray(x, dtype=np.float32)
    conv_w = np.asarray(conv_w, dtype=np.float32)
    W1 = np.asarray(W1, dtype=np.float32)
    b1 = np.asarray(b1, dtype=np.float32)
    W2 = np.asarray(W2, dtype=np.float32)
    b2 = np.asarray(b2, dtype=np.float32)
    W3 = np.asarray(W3, dtype=np.float32)
    b3 = np.asarray(b3, dtype=np.float32)

    C = _conv_unfold(conv_w)
    W1f = np.zeros((K1P, M1), dtype=NP_DT)  # K padded 784 -> 896
    W1f[:K1] = (C @ W1.astype(np.float64)).astype(NP_DT)
    W2q = W2.astype(NP_DT)
    # W3 padded: k-tiles 100 -> 128 rows, output 10 -> 128 cols
    W3p = np.zeros((K3T, NK3, M3P), dtype=NP_DT)
    for k in range(NK3):
        W3p[:M2T, k, :M3] = W3[k * M2T:(k + 1) * M2T].astype(NP_DT)

    nc = _get_nc()
    xT = x.T  # (784, 65536) view
    in_maps = []
    for c in range(NCORES):
        xh = np.zeros((NK1 * K1T, BC), dtype=NP_DT)
        xh[:K1] = xT[:, c * BC:(c + 1) * BC]
        in_maps.append({
            "xh": xh.reshape(NK1, K1T, BC), "w1": W1f, "b1": b1,
            "w2": W2q, "b2": b2, "w3": W3p, "b3": b3,
        })

    res = run_bass_kernel_spmd(
        nc, in_maps, list(range(NCORES)), trace=_trace, tmpdir=_tmpdir)
    out = np.empty((B, M3), dtype=np.float32)
    for c in range(NCORES):
        out[c * BC:(c + 1) * BC, :] = res.results[c]["out"].T
    if _trace:
        return out, res
    return out
